# revision 19
# baseline (speedup 1.0000x reference)
"""Chamfer distance L2 (mean-compressed) on 8 Trainium2 NeuronCores.

Sharding: data-parallel over batch B=16 -> 2 batches per core; each core
computes partial min-distance reductions; the host finishes the fold and
averages (the "all-reduce" of the mean).

Per batch on one core the kernel computes the negated squared-distance matrix
    -d[i, j] = 2 p_i . g_j - |p_i|^2 - |g_j|^2
on the tensor engine.  To get fp32-grade accuracy at full bf16 PE rate,
every fp32 operand is split into 3 bf16 levels (x = x0 + x1 + x2,
residual ~2^-27) and the K=5 augmented-point contraction is expanded into
K=24 bf16 rows covering all product pairs down to 2^-27 (see _augment).
PSUM accumulates in fp32; device values are -512*d.

Reduction strategy (the baseline was ScalarE-copy bound at ~258us): the
per-ptile [128 x 4096] PSUM block is split into two [128 x 2048] halves with
the evacuation load spread over three engines so no single engine sees more
than ~2us per ptile:
  half A: ScalarE TensorCopy -> fp16 SBUF; VectorE tensor_scalar (fp16 4x
          mode) folds the row-min into rowm and re-materializes nothing.
  half B: GpSimd (Pool engine) tensor_scalar straight from PSUM with fused
          row-min accumulation, writing the fp16 SBUF copy as its output.
  col-min (dist2): VectorE tensor_tensor max-accumulate on the fp16 copies
          (4x perf mode), per gt-column accumulator tiles.
The per-column accumulators (dist2) and per-(ptile,half) row maxes (dist1)
are DMA'd out raw; the host does the final tiny folds in numpy.
"""

import numpy as np

_B, _N, _M = 16, 4096, 4096
_NCORES = 8
_BPC = _B // _NCORES  # batches per core
_PT = _N // 128       # pred tiles per batch
_HALF = 2048          # gt columns per psum group (4 banks)
_K = 24               # split-contraction depth
_SCALE = 512.0        # device values are -_SCALE * d

_cache = None


# Schedule config.  PSUM is tiled as four independent [128, 1024] quarters
# (2 banks each) so the WAR recycle pipeline runs at quarter granularity.
#   evac[q]: "act" (ScalarE copy) | "dve" (VectorE TS fused with row-min) |
#            "alt" (Act on even ptiles, DVE on odd)
#       Act quarters must be a prefix (their sb region is reduced by one
#       contiguous row-min TS on DVE).
#   dve_tt_spill: trailing columns of the B colmax DMA moved to a DVE TT.
_DEFAULT_SCHED = {
    "evac": ["act", "act", "alt", "dve"],
    "spill": {"dve": 512, "pool": 512},
    "sb_bufs": 7,
    "nchain": 2,
    "work_bufs": 4,
}


def _build_nc(sched=_DEFAULT_SCHED):
    import concourse.mybir as mybir
    from concourse import tile, bacc

    dt = mybir.dt
    Alu = mybir.AluOpType
    f32, bf16, f16 = dt.float32, dt.bfloat16, dt.float16

    nc = bacc.Bacc("TRN2", target_bir_lowering=False, debug=False)

    def act_copy(out, in_):
        # Plain copy pinned on ScalarE (walrus rejects TensorTensor /
        # TensorScalar / TensorReduce on Activation for TRN2, but TensorCopy
        # is fine).
        eng = nc.scalar
        return eng.add_instruction(
            mybir.InstTensorCopy(
                name=f"I-{nc.next_id()}",
                ins=[eng.lower_ap(in_)],
                outs=[eng.lower_ap(out)],
            )
        )

    predA = nc.dram_tensor("predA", [_K, _BPC * _N], bf16, kind="ExternalInput").ap()
    gtA = nc.dram_tensor("gtA", [_K, _BPC * _M], bf16, kind="ExternalInput").ap()
    # row maxes of -512*d: three columns per (batch, ptile):
    # [act-prefix, fused-q2, fused-q3]; host max-folds them
    rowm_d = nc.dram_tensor(
        "rowmins", [128, _BPC * _PT * 3], f32, kind="ExternalOutput"
    ).ap()
    # per-column accumulators (max over all pred rows seen per partition):
    # [batch*half*128, 2048] fp16, host folds the 128 partitions
    colm_d = nc.dram_tensor(
        "colmins", [_BPC * 128, 2 * _HALF], f16, kind="ExternalOutput"
    ).ap()

    nchain = sched["nchain"]
    spill = sched["spill"]
    _Q = 1024

    with tile.TileContext(nc) as tc:
        with (
            tc.tile_pool(name="io", bufs=1) as io,
            tc.tile_pool(name="dcp", bufs=sched["sb_bufs"]) as dcp,
            tc.tile_pool(name="acc", bufs=1) as acc,
            tc.tile_pool(name="work", bufs=sched.get("work_bufs", 2)) as work,
            tc.tile_pool(name="ps", bufs=1, space="PSUM") as ps,
        ):
            pa = io.tile([_K, _BPC * _N], bf16, tag="pa")
            ga = io.tile([_K, _BPC * _M], bf16, tag="ga")
            # split input loads per batch so batch 0 compute overlaps the
            # batch 1 load
            for b in range(_BPC):
                nc.sync.dma_start(pa[:, b * _N : (b + 1) * _N],
                                  predA[:, b * _N : (b + 1) * _N])
                nc.sync.dma_start(ga[:, b * _M : (b + 1) * _M],
                                  gtA[:, b * _M : (b + 1) * _M])
            rowm = io.tile([128, _BPC * _PT * 3], f32, tag="rowm")
            # even ptiles write a single fused row-min column; flood the
            # unused odd columns so the host's max-fold ignores them
            nc.gpsimd.memset(rowm[:], -3.0e38)

            for b in range(_BPC):
                # nchain interleaved accumulators (full 4096 gt columns) to
                # shorten the serial read-modify-write chains
                cols = [
                    acc.tile(
                        [128, 2 * _HALF], f16,
                        tag=f"col{b}_{k}", name=f"col{b}_{k}",
                    )
                    for k in range(nchain)
                ]
                for p in range(_PT):
                    psq = [
                        ps.tile([128, _Q], f32, tag=f"ps{q}", name=f"ps{q}")
                        for q in range(4)
                    ]
                    lp = b * _N + p * 128
                    lhsT = pa[:, lp : lp + 128]
                    for q in range(4):
                        for s in range(2):
                            c0 = b * _M + q * _Q + s * 512
                            nc.tensor.matmul(
                                psq[q][:, s * 512 : (s + 1) * 512],
                                lhsT,
                                ga[:, c0 : c0 + 512],
                                start=True,
                                stop=True,
                            )
                    col = (b * _PT + p) * 3
                    # fp16 staging: one [128, 4096] tile in gt order; quarters
                    # may be written by different engines (subtile deps)
                    sb = dcp.tile([128, 2 * _HALF], f16, tag="sb")
                    on_dve = [
                        e == "dve" or (e == "alt" and p % 2 == 1)
                        for e in sched["evac"]
                    ]
                    n_act = sum(1 for x in on_dve if not x)
                    assert on_dve == sorted(on_dve), "act quarters must prefix"
                    for q in range(4):
                        sbq = sb[:, q * _Q : (q + 1) * _Q]
                        if on_dve[q]:
                            # fused evac + row-min accum on DVE (PSUM 1x)
                            nc.vector.tensor_scalar(
                                sbq,
                                psq[q][:],
                                -65504.0,
                                None,
                                op0=Alu.max,
                                op1=Alu.max,
                                accum_out=rowm[:, col + q - 1 : col + q],
                            )
                        else:
                            act_copy(sbq, psq[q][:])
                    # row-min of the contiguous Act prefix (fp16 4x mode);
                    # lands in a column that the host max-folds with the
                    # fused columns
                    if n_act:
                        dummy = work.tile([128, 3 * _Q], f16, tag="dum")
                        nc.vector.tensor_scalar(
                            dummy[:, 0 : n_act * _Q],
                            sb[:, 0 : n_act * _Q],
                            -65504.0,
                            None,
                            op0=Alu.max,
                            op1=Alu.max,
                            accum_out=rowm[:, col : col + 1],
                        )
                    # --- dist2 col-max accumulate ---
                    # max is idempotent/commutative, so accumulation order
                    # across engines doesn't matter; Tile serializes writers
                    # of each accumulator, hence the chain split.
                    ck = cols[p % nchain]
                    W = 2 * _HALF
                    dmaw = W - spill["dve"] - spill["pool"]
                    if p < nchain:
                        ckh = ck[:, 0:_HALF]
                        nc.vector.tensor_scalar(
                            ckh, sb[:, 0:_HALF], -65504.0, None, op0=Alu.max
                        )
                        ckh2 = ck[:, _HALF:W]
                        nc.vector.tensor_scalar(
                            ckh2, sb[:, _HALF:W], -65504.0, None, op0=Alu.max
                        )
                    else:
                        nc.gpsimd.dma_start(
                            ck[:, 0:dmaw], sb[:, 0:dmaw], accum_op=Alu.max
                        )
                        if spill["dve"]:
                            d0, d1 = dmaw, dmaw + spill["dve"]
                            nc.vector.tensor_tensor(
                                ck[:, d0:d1], sb[:, d0:d1], ck[:, d0:d1],
                                op=Alu.max,
                            )
                        if spill["pool"]:
                            p0 = dmaw + spill["dve"]
                            nc.gpsimd.tensor_tensor(
                                ck[:, p0:W], sb[:, p0:W], ck[:, p0:W],
                                op=Alu.max,
                            )
                for k in range(1, nchain):
                    nc.vector.tensor_tensor(
                        cols[0][:], cols[k][:], cols[0][:], op=Alu.max
                    )
                nc.sync.dma_start(
                    colm_d[b * 128 : (b + 1) * 128, :], cols[0][:]
                )
            nc.sync.dma_start(rowm_d[:], rowm[:])
    nc.compile()
    return nc


def _get_runtime():
    """Build the Bass program once and wrap it in a cached sharded jit
    (mirrors bass2jax.run_bass_via_pjrt's multi-core branch so repeated
    kernel() calls reuse the compiled NEFF)."""
    global _cache
    if _cache is not None:
        return _cache

    import jax
    from jax.experimental.shard_map import shard_map
    from jax.sharding import Mesh, PartitionSpec
    import concourse.mybir as mybir
    from concourse import bass2jax

    nc = _build_nc()
    bass2jax.install_neuronx_cc_hook()

    partition_name = nc.partition_id_tensor.name if nc.partition_id_tensor else None
    in_names, out_names, out_avals = [], [], []
    for alloc in nc.m.functions[0].allocations:
        if not isinstance(alloc, mybir.MemoryLocationSet):
            continue
        name = alloc.memorylocations[0].name
        if alloc.kind == "ExternalInput":
            if name != partition_name:
                in_names.append(name)
        elif alloc.kind == "ExternalOutput":
            out_names.append(name)
            out_avals.append(
                jax.core.ShapedArray(
                    tuple(alloc.tensor_shape), mybir.dt.np(alloc.dtype)
                )
            )
    n_params = len(in_names)
    n_outs = len(out_avals)
    all_in_names = list(in_names) + list(out_names)
    if partition_name is not None:
        all_in_names.append(partition_name)

    def _body(*args):
        operands = list(args)
        if partition_name is not None:
            operands.append(bass2jax.partition_id_tensor())
        outs = bass2jax._bass_exec_p.bind(
            *operands,
            out_avals=tuple(out_avals),
            in_names=tuple(all_in_names),
            out_names=tuple(out_names),
            lowering_input_output_aliases=(),
            sim_require_finite=True,
            sim_require_nnan=True,
            nc=nc,
        )
        return tuple(outs)

    devices = jax.devices()[:_NCORES]
    assert len(devices) == _NCORES, f"need {_NCORES} cores, got {len(jax.devices())}"
    mesh = Mesh(np.asarray(devices), ("core",))
    in_specs = (PartitionSpec("core"),) * (n_params + n_outs)
    out_specs = (PartitionSpec("core"),) * n_outs
    donate = tuple(range(n_params, n_params + n_outs))
    sharded = jax.jit(
        shard_map(
            _body, mesh=mesh, in_specs=in_specs, out_specs=out_specs, check_rep=False
        ),
        donate_argnums=donate,
        keep_unused=True,
    )
    _cache = (sharded, in_names, out_names, out_avals)
    return _cache


def _split3(x):
    """fp32 -> 3 bf16 levels whose sum reproduces x to ~2^-27 relative."""
    import ml_dtypes

    bf = ml_dtypes.bfloat16
    x0 = x.astype(bf)
    r = x - x0.astype(np.float32)
    x1 = r.astype(bf)
    r -= x1.astype(np.float32)
    x2 = r.astype(bf)
    return x0, x1, x2


def _augment(prediction, gt):
    """Host-side prep: bf16 split-augmented matrices [B, 24, N]/[B, 24, M].

    (lhsT.T @ rhs)[i, j] = 2 p.g - |p|^2 - |g|^2 = -d[i, j]
    """
    import ml_dtypes

    bf = ml_dtypes.bfloat16
    pred = np.asarray(prediction, dtype=np.float32)
    g = np.asarray(gt, dtype=np.float32)
    p2 = np.sum(pred * pred, axis=-1)  # [B, N]
    g2 = np.sum(g * g, axis=-1)  # [B, M]

    predA = np.empty((_B, _K, _N), bf)
    gtA = np.empty((_B, _K, _M), bf)
    for d in range(3):
        pd0, pd1, pd2 = _split3(pred[:, :, d])
        Gd0, Gd1, Gd2 = _split3(2.0 * g[:, :, d])
        base = d * 6
        # product pairs (0,0),(0,1),(1,0),(1,1),(0,2),(2,0)
        for r, (pi, gi) in enumerate(
            [(0, 0), (0, 1), (1, 0), (1, 1), (0, 2), (2, 0)]
        ):
            predA[:, base + r, :] = (pd0, pd1, pd2)[pi]
            gtA[:, base + r, :] = (Gd0, Gd1, Gd2)[gi]
    q0, q1, q2 = _split3(p2)
    r0, r1, r2 = _split3(g2)
    for lvl, q in enumerate((q0, q1, q2)):
        predA[:, 18 + lvl, :] = q
        gtA[:, 18 + lvl, :] = bf(-1.0)
    for lvl, r in enumerate((r0, r1, r2)):
        predA[:, 21 + lvl, :] = bf(1.0)
        gtA[:, 21 + lvl, :] = -r
    # scale the product by 2^9 (16 * 32, exact in bf16) so the fp16 min
    # stage stays far from subnormals: device values are -512*d
    predA = (predA.astype(np.float32) * 16.0).astype(bf)
    gtA = (gtA.astype(np.float32) * 32.0).astype(bf)
    return predA, gtA


def kernel(prediction, gt):
    sharded, in_names, out_names, out_avals = _get_runtime()

    predA, gtA = _augment(prediction, gt)
    # per-core inputs: batches [c*BPC, (c+1)*BPC) concatenated column-wise
    per_core = {
        "predA": [
            predA[c * _BPC : (c + 1) * _BPC]
            .transpose(1, 0, 2)
            .reshape(_K, _BPC * _N)
            for c in range(_NCORES)
        ],
        "gtA": [
            gtA[c * _BPC : (c + 1) * _BPC].transpose(1, 0, 2).reshape(_K, _BPC * _M)
            for c in range(_NCORES)
        ],
    }
    concat_in = [
        np.ascontiguousarray(np.concatenate(per_core[name], axis=0))
        for name in in_names
    ]
    concat_zeros = [
        np.zeros((_NCORES * a.shape[0],) + tuple(a.shape[1:]), a.dtype)
        for a in out_avals
    ]
    out_arrs = sharded(*concat_in, *concat_zeros)

    outs = {name: np.asarray(out_arrs[i]) for i, name in enumerate(out_names)}
    # rowmins: [8*128, BPC*PT*2] f32 of -512*d row maxes; every pred point's
    # min distance is -max(row)/512, with the two half-columns folded
    rowm = outs["rowmins"].reshape(_NCORES, 128, _BPC * _PT * 2)
    rowmax = np.max(
        rowm.reshape(_NCORES, 128, _BPC * _PT, 2), axis=3
    )  # [cores, 128, BPC*PT]
    sum1 = -np.sum(rowmax.astype(np.float64)) / _SCALE

    # colmins: [8 * BPC*2*128, 2048] f16: per-partition col maxes; fold the
    # 128 partitions then sum
    colm = outs["colmins"].astype(np.float32).reshape(_NCORES * _BPC, 128, 2 * _HALF)
    colmax = np.max(colm, axis=1)  # [cores*BPC*2, 2048]
    sum2 = -np.sum(colmax.astype(np.float64)) / _SCALE

    result = (sum1 + sum2) / float(_B * _N)
    return np.float32(result)
